# revision 1
# baseline (speedup 1.0000x reference)
"""DetectionLoss kernel for Trainium2 (Bass/Tile), 8-core data parallel.

Problem: B=16 images, P=16384 predicted boxes, T=128 true boxes, C=80 classes.
  bbox_loss = sum(smooth_l1(pred - matched_true) * (max_iou > 0.5)) / max(4*n_matched, 1)
  cls_loss  = -mean over B of log_softmax(pred_classes[:,0,:])[label[:,0]]
  out       = bbox_loss + cls_loss   (f32 scalar)

Sharding: batch dim across 8 cores (2 images per core). Each core returns
per-partition partial sums (bbox sums, match counts, cls NLL); the host
combines them into the final scalar.

Device algorithm (per image), with T=128 on the free dim and 128 preds per
partition-chunk, G=8 chunks per instruction via 0-stride "repeat" APs:
  * pairwise intersection inter[p,t] = relu(min(x2) - max(x1)) * relu(... y)
  * IoU ordering via the int-log2 trick: for positive f32, the int32 bit
    pattern is a monotone (piecewise-linear) map of log2(x). So
    lwi = int(inter) - int(pa+ta) orders pairs like log2(IoU surrogate
    w = inter/(pa+ta)), and IoU>0.5 <=> w>1/3 <=> lwi > ~ -1.585*2^23.
    The approximation wobbles the 0.5 threshold within ~[0.47,0.53] and can
    flip argmax between near-ties; both effects are ~1e-6 of the total loss
    (cls_loss ~ 4.9 dominates; bbox_loss ~ 2e-4).
  * matched smooth-l1 (|d|<1 always holds for IoU>0.5 pairs, so sl1 = d^2/2):
    sum_c d^2 = P2[p] + (q[t] - 2*pred.tb[t])|_{t=argmax}, where the bilinear
    term r2[p,t] = pred[p,:].(2*tb[t,:]) is a K=4 matmul on the PE, and the
    argmax selection is a one-hot multiply + segmented reduce.
"""

import numpy as np

import concourse.bacc as bacc
import concourse.bass as bass
import concourse.tile as tile
from concourse import mybir
from concourse.bass_utils import run_bass_kernel_spmd

F32 = mybir.dt.float32
I32 = mybir.dt.int32
ALU = mybir.AluOpType
ACTF = mybir.ActivationFunctionType
AXX = mybir.AxisListType.X

B, P_TOT, T, C = 16, 16384, 128, 80
NCORES = 8
NIMG = B // NCORES          # images per core
NP = 128                    # partitions
NCH = P_TOT // NP           # chunks per image (chunk = 128 preds)
G = 8                       # chunks per instruction
NSC = NCH // G              # super-chunks per image
# int-log2 threshold: lw > log2(1/3) * 2^23
ITHRESH = float(np.round(np.log2(1.0 / 3.0) * (1 << 23)))


def _rep_b(t, g=G):
    """[128, T] tile -> [128, g, T] AP, repeated across chunks."""
    return bass.AP(tensor=t.tensor, offset=t.offset, ap=[t.ap[0], [0, g], list(t.ap[1])])


def _rep_s(t, g=G):
    """[128, g] strided column slice -> [128, g, T] AP, repeated across t."""
    return bass.AP(tensor=t.tensor, offset=t.offset, ap=[t.ap[0], list(t.ap[1]), [0, T]])


def build_nc():
    nc = bacc.Bacc("TRN2", target_bir_lowering=False, debug=False)

    pred_d = nc.declare_dram_parameter("pred", [NIMG, P_TOT, 4], F32, isOutput=False)
    predT_d = nc.declare_dram_parameter("predT", [NIMG, 4, P_TOT], F32, isOutput=False)
    tbT_d = nc.declare_dram_parameter("tbT", [NIMG, 4, T], F32, isOutput=False)
    logits_d = nc.declare_dram_parameter("logits", [NIMG, C], F32, isOutput=False)
    oh80_d = nc.declare_dram_parameter("oh80", [NIMG, C], F32, isOutput=False)
    out_d = nc.declare_dram_parameter("out", [NP, 8], F32, isOutput=True)

    with tile.TileContext(nc) as tc:
        consts = tc.alloc_tile_pool(name="consts", bufs=1)
        imgp = tc.alloc_tile_pool(name="imgp", bufs=2)
        chkp = tc.alloc_tile_pool(name="chkp", bufs=2)
        psp = tc.alloc_tile_pool(name="psp", bufs=2, space="PSUM")

        out_sb = consts.tile([NP, 8], F32)
        nc.vector.memset(out_sb, 0.0)

        # ---------------- classification loss (tiny) ----------------
        logit_sb = consts.tile([NIMG, C], F32)
        nc.sync.dma_start(out=logit_sb, in_=logits_d.ap())
        oh_sb = consts.tile([NIMG, C], F32)
        nc.sync.dma_start(out=oh_sb, in_=oh80_d.ap())

        mx = consts.tile([NIMG, 1], F32)
        nc.vector.tensor_reduce(mx, logit_sb, AXX, ALU.max)
        zc = consts.tile([NIMG, C], F32)
        nc.vector.tensor_scalar(zc, logit_sb, mx, None, ALU.subtract)
        ez = consts.tile([NIMG, C], F32)
        se = consts.tile([NIMG, 1], F32)
        nc.scalar.activation(ez, zc, ACTF.Exp, accum_out=se)
        lnse = consts.tile([NIMG, 1], F32)
        nc.scalar.activation(lnse, se, ACTF.Ln)
        zl = consts.tile([NIMG, 1], F32)
        zprod = consts.tile([NIMG, C], F32)
        nc.vector.tensor_tensor(zprod, zc, oh_sb, ALU.mult)
        nc.vector.tensor_reduce(zl, zprod, AXX, ALU.add)
        # nll = lnse - (z_label - mx) = lse - z_label
        nc.vector.tensor_tensor(out_sb[0:NIMG, 4:5], lnse, zl, ALU.subtract)

        # ---------------- bbox loss ----------------
        for img in range(NIMG):
            # pred laid out [p, n, coord] with row = n*128 + p (chunk-major,
            # matching the PE matmul's output-partition = row-within-chunk).
            pred_sb = imgp.tile([NP, NCH, 4], F32, tag="pred")
            pred_img = pred_d.ap()[img].rearrange("(n p) c -> p n c", p=NP)
            nc.sync.dma_start(out=pred_sb, in_=pred_img)

            # tbT natural [4, T] (+ doubled copy for the bilinear matmul)
            tbT_sb = imgp.tile([4, T], F32, tag="tbT")
            nc.sync.dma_start(out=tbT_sb, in_=tbT_d.ap()[img])
            tbT2_sb = imgp.tile([4, T], F32, tag="tbT2")
            nc.vector.tensor_scalar(tbT2_sb, tbT_sb, 2.0, None, ALU.mult)

            # broadcast tiles: every partition holds the t-row of each coord
            tbT_img = tbT_d.ap()[img]
            bt = []
            for coord in range(4):
                btile = imgp.tile([NP, T], F32, tag=f"bt{coord}")
                src = bass.AP(
                    tensor=tbT_img.tensor,
                    offset=tbT_img.offset + coord * T,
                    ap=[[0, NP], [1, T]],
                )
                nc.gpsimd.dma_start(out=btile, in_=src)
                bt.append(btile)
            tx1b, ty1b, tx2b, ty2b = bt

            # true-box area and squared-norm broadcast tiles
            tw = imgp.tile([NP, T], F32, tag="tw")
            nc.vector.tensor_tensor(tw, tx2b, tx1b, ALU.subtract)
            th = imgp.tile([NP, T], F32, tag="th")
            nc.vector.tensor_tensor(th, ty2b, ty1b, ALU.subtract)
            taB = imgp.tile([NP, T], F32, tag="taB")
            nc.vector.tensor_tensor(taB, tw, th, ALU.mult)

            q1 = imgp.tile([NP, T], F32, tag="q1")
            nc.gpsimd.tensor_tensor(q1, tx1b, tx1b, ALU.mult)
            q2 = imgp.tile([NP, T], F32, tag="q2")
            nc.gpsimd.tensor_tensor(q2, ty1b, ty1b, ALU.mult)
            q3 = imgp.tile([NP, T], F32, tag="q3")
            nc.gpsimd.tensor_tensor(q3, tx2b, tx2b, ALU.mult)
            q4 = imgp.tile([NP, T], F32, tag="q4")
            nc.gpsimd.tensor_tensor(q4, ty2b, ty2b, ALU.mult)
            q12 = imgp.tile([NP, T], F32, tag="q12")
            nc.gpsimd.tensor_tensor(q12, q1, q2, ALU.add)
            q34 = imgp.tile([NP, T], F32, tag="q34")
            nc.gpsimd.tensor_tensor(q34, q3, q4, ALU.add)
            qB = imgp.tile([NP, T], F32, tag="qB")
            nc.gpsimd.tensor_tensor(qB, q12, q34, ALU.add)

            # pred areas (clamped >= 0: inverted jittered boxes have
            # inter == 0 everywhere, and a negative pa would corrupt the
            # int-log2 of pa+ta) and pred squared-norms, per chunk column
            pw = imgp.tile([NP, NCH], F32, tag="pw")
            nc.vector.tensor_tensor(pw, pred_sb[:, :, 2], pred_sb[:, :, 0], ALU.subtract)
            ph = imgp.tile([NP, NCH], F32, tag="ph")
            nc.vector.tensor_tensor(ph, pred_sb[:, :, 3], pred_sb[:, :, 1], ALU.subtract)
            paRaw = imgp.tile([NP, NCH], F32, tag="paRaw")
            nc.vector.tensor_tensor(paRaw, pw, ph, ALU.mult)
            paAll = imgp.tile([NP, NCH], F32, tag="paAll")
            nc.vector.tensor_scalar(paAll, paRaw, 0.0, None, ALU.max)

            psq = imgp.tile([NP, NCH, 4], F32, tag="psq")
            nc.vector.tensor_tensor(psq, pred_sb, pred_sb, ALU.mult)
            p12 = imgp.tile([NP, NCH], F32, tag="p12")
            nc.vector.tensor_tensor(p12, psq[:, :, 0], psq[:, :, 1], ALU.add)
            p34 = imgp.tile([NP, NCH], F32, tag="p34")
            nc.vector.tensor_tensor(p34, psq[:, :, 2], psq[:, :, 3], ALU.add)
            P2All = imgp.tile([NP, NCH], F32, tag="P2All")
            nc.vector.tensor_tensor(P2All, p12, p34, ALU.add)

            maxiAll = imgp.tile([NP, NCH], I32, tag="maxiAll")
            uamAll = imgp.tile([NP, NCH], F32, tag="uamAll")

            for sc in range(NSC):
                c0 = sc * G
                cols = slice(c0, c0 + G)
                px1 = _rep_s(pred_sb[:, cols, 0])
                py1 = _rep_s(pred_sb[:, cols, 1])
                px2 = _rep_s(pred_sb[:, cols, 2])
                py2 = _rep_s(pred_sb[:, cols, 3])

                # x-axis interval on DVE
                a_t = chkp.tile([NP, G, T], F32, tag="a")
                nc.vector.tensor_tensor(a_t, _rep_b(tx2b), px2, ALU.min)
                mxx = chkp.tile([NP, G, T], F32, tag="mxx")
                nc.vector.tensor_tensor(mxx, _rep_b(tx1b), px1, ALU.max)
                dx = chkp.tile([NP, G, T], F32, tag="dx")
                nc.vector.tensor_tensor(dx, a_t, mxx, ALU.subtract)
                rdx = chkp.tile([NP, G, T], F32, tag="rdx")
                nc.scalar.activation(rdx, dx, ACTF.Relu)

                # y-axis interval: min/max on DVE (Pool lacks min/max),
                # subtract on GPSIMD
                b_t = chkp.tile([NP, G, T], F32, tag="b")
                nc.vector.tensor_tensor(b_t, _rep_b(ty2b), py2, ALU.min)
                mxy = chkp.tile([NP, G, T], F32, tag="mxy")
                nc.vector.tensor_tensor(mxy, _rep_b(ty1b), py1, ALU.max)
                dy = chkp.tile([NP, G, T], F32, tag="dy")
                nc.gpsimd.tensor_tensor(dy, b_t, mxy, ALU.subtract)
                rdy = chkp.tile([NP, G, T], F32, tag="rdy")
                nc.scalar.activation(rdy, dy, ACTF.Relu)

                inter = chkp.tile([NP, G, T], F32, tag="inter")
                nc.gpsimd.tensor_tensor(inter, rdx, rdy, ALU.mult)
                s_t = chkp.tile([NP, G, T], F32, tag="s")
                nc.vector.tensor_tensor(s_t, _rep_b(taB), _rep_s(paAll[:, cols]), ALU.add)

                # int-log2 ordering + segmented argmax
                lwi = chkp.tile([NP, G, T], I32, tag="lwi")
                nc.vector.tensor_tensor(lwi, inter.bitcast(I32), s_t.bitcast(I32), ALU.subtract)
                nc.vector.tensor_reduce(maxiAll[:, cols], lwi, AXX, ALU.max)
                oh_t = chkp.tile([NP, G, T], F32, tag="oh")
                nc.vector.tensor_tensor(
                    oh_t, lwi, _rep_s(maxiAll[:, cols]).bitcast(I32), ALU.is_equal
                )

                # bilinear term r2[p,t] = pred . (2 tb): K=4 matmuls on PE
                predT_sc = chkp.tile([4, G * NP], F32, tag="predT")
                src = bass.AP(
                    tensor=predT_d.ap().tensor,
                    offset=predT_d.ap().offset + img * 4 * P_TOT + c0 * NP,
                    ap=[[P_TOT, 4], [1, G * NP]],
                )
                nc.sync.dma_start(out=predT_sc, in_=src)
                r2_ps = psp.tile([NP, G, T], F32, tag="r2")
                for k in range(G):
                    nc.tensor.matmul(
                        r2_ps[:, k, :],
                        predT_sc[:, k * NP : (k + 1) * NP],
                        tbT2_sb,
                        start=True,
                        stop=True,
                    )

                # u = q - 2 r ; select at argmax
                u_t = chkp.tile([NP, G, T], F32, tag="u")
                nc.vector.tensor_tensor(u_t, _rep_b(qB), r2_ps, ALU.subtract)
                usel = chkp.tile([NP, G, T], F32, tag="usel")
                nc.gpsimd.tensor_tensor(usel, oh_t, u_t, ALU.mult)
                nc.vector.tensor_reduce(uamAll[:, cols], usel, AXX, ALU.add)

            # image tail
            maskAll = imgp.tile([NP, NCH], F32, tag="maskAll")
            nc.vector.tensor_scalar(maskAll, maxiAll, ITHRESH, None, ALU.is_gt)
            g_t = imgp.tile([NP, NCH], F32, tag="g")
            nc.vector.tensor_tensor(g_t, P2All, uamAll, ALU.add)
            csum = imgp.tile([NP, NCH], F32, tag="csum")
            nc.vector.tensor_tensor(csum, g_t, maskAll, ALU.mult)

            nc.vector.tensor_reduce(out_sb[:, img : img + 1], csum, AXX, ALU.add)
            nc.vector.tensor_reduce(out_sb[:, 2 + img : 3 + img], maskAll, AXX, ALU.add)

        nc.sync.dma_start(out=out_d.ap(), in_=out_sb)

        for p in (psp, chkp, imgp, consts):
            p.release()

    nc.compile()
    return nc


_NC_CACHE = None


def _get_nc():
    global _NC_CACHE
    if _NC_CACHE is None:
        _NC_CACHE = build_nc()
    return _NC_CACHE


def make_in_maps(pred_bboxes, pred_classes, true_bboxes, true_labels):
    pred_bboxes = np.ascontiguousarray(pred_bboxes, dtype=np.float32)
    true_bboxes = np.ascontiguousarray(true_bboxes, dtype=np.float32)
    logits0 = np.ascontiguousarray(pred_classes[:, 0, :], dtype=np.float32)
    lab0 = np.asarray(true_labels)[:, 0].astype(np.int64)
    oh80 = np.zeros((B, C), dtype=np.float32)
    oh80[np.arange(B), lab0] = 1.0

    in_maps = []
    for c in range(NCORES):
        s = slice(c * NIMG, (c + 1) * NIMG)
        in_maps.append(
            {
                "pred": pred_bboxes[s],
                "predT": np.ascontiguousarray(pred_bboxes[s].transpose(0, 2, 1)),
                "tbT": np.ascontiguousarray(true_bboxes[s].transpose(0, 2, 1)),
                "logits": logits0[s],
                "oh80": oh80[s],
            }
        )
    return in_maps


def combine(outs):
    bbox_sum = 0.0
    n_matched = 0.0
    cls_sum = 0.0
    for o in outs:
        o64 = o.astype(np.float64)
        bbox_sum += o64[:, 0:NIMG].sum()
        n_matched += o64[:, NIMG : 2 * NIMG].sum()
        cls_sum += o64[0:NIMG, 4].sum()
    bbox_loss = 0.5 * bbox_sum / max(4.0 * n_matched, 1.0)
    cls_loss = cls_sum / B
    return np.float32(bbox_loss + cls_loss)


def run_device(in_maps, trace=False, **kwargs):
    nc = _get_nc()
    return run_bass_kernel_spmd(
        nc, in_maps, list(range(NCORES)), trace=trace, **kwargs
    )


def kernel(pred_bboxes, pred_classes, true_bboxes, true_labels):
    in_maps = make_in_maps(pred_bboxes, pred_classes, true_bboxes, true_labels)
    res = run_device(in_maps)
    outs = [res.results[i]["out"] for i in range(NCORES)]
    return combine(outs)



# revision 13
# speedup vs baseline: 1.4676x; 1.4676x over previous
"""DetectionLoss kernel for Trainium2 (Bass/Tile), 8-core data parallel.

Problem: B=16 images, P=16384 predicted boxes, T=128 true boxes, C=80 classes.
  bbox_loss = sum(smooth_l1(pred - matched_true) * (max_iou > 0.5)) / max(4*n_matched, 1)
  cls_loss  = -mean over B of log_softmax(pred_classes[:,0,:])[label[:,0]]
  out       = bbox_loss + cls_loss   (f32 scalar)

Sharding: batch dim across 8 cores (2 images per core). Each core returns
per-partition partial sums (d^2 sums, match counts, cls NLL); the host
combines them into the final scalar.

Device algorithm v2 (bf16 pairwise + fused DVE ops + PE block-diag matmuls):
  * Layout: partitions = 128 preds of a chunk; free dims = (t=128, g=8 chunks),
    g innermost so every DVE operand streams with innermost step 1 and 16-bit
    dtype -> 2x DVE mode. True-box tiles are materialized [128, T, G] (value
    depends on t only); pred coords are read as [.., 0-stride T, chunk-window].
  * inter = relu(min(px2,tx2)-max(px1,tx1)) * relu(...y...): min/max on DVE
    (bf16 2x), the subtracts on GPSIMD, relu(dy) on Scalar, and
    relu(dx)*rdy fused in one DVE scalar_tensor_tensor.
  * Ordering via the bf16 int-log trick: key = i16(inter) - i16(pa+ta) orders
    pairs like log2(inter/(pa+ta)), which is IoU-monotone per pred.
    IoU>0.5 <=> key > log2(1/3)*2^7 ~ -203. bf16 quantization wobbles the
    threshold/argmax within ~3%, worth ~1e-6 of the total loss (cls ~4.9
    dominates; bbox ~2e-4).
  * s = pa+ta and u = |pred - tb|^2 come from the PE as single block-diagonal
    matmuls per super-chunk (8 chunks' weights stacked as 48/16 lhsT rows,
    block-diag rhs built on host), Scalar-copied PSUM->SBUF as bf16.
    u must be computed in f32 on the PE (catastrophic cancellation:
    d^2 ~ 4e-4 while P2+qB ~ 2).
  * Selection fused on DVE: rmax = tensor_reduce(key) per chunk; rmaxP =
    (rmax>TH ? rmax : 32767) [never equals a real key]; then
    tensor_tensor_reduce twice: oh = (key==rmaxP-bcast) with accum -> count,
    ohu = oh*u with accum -> d^2 sum. Below-threshold preds select nothing.
"""

import os
import numpy as np
import ml_dtypes

import concourse.bacc as bacc
import concourse.bass as bass
import concourse.tile as tile
from concourse import mybir
from concourse.bass_utils import run_bass_kernel_spmd

F32 = mybir.dt.float32
BF16 = mybir.dt.bfloat16
I16 = mybir.dt.int16
I32 = mybir.dt.int32
ALU = mybir.AluOpType
ACTF = mybir.ActivationFunctionType
AXX = mybir.AxisListType.X

B, P_TOT, T, C = 16, 16384, 128, 80
NCORES = 8
NIMG = B // NCORES          # images per core
NP = 128                    # partitions
NCH = P_TOT // NP           # chunks per image (chunk = 128 preds)
G = 8                       # chunks per instruction
NSC = NCH // G              # super-chunks per image
SCW = T * G                 # free elements per instruction
# bf16 int-log threshold: key > log2(1/3) * 2^7
ITH16 = int(np.round(np.log2(1.0 / 3.0) * (1 << 7)))  # -203
BF = ml_dtypes.bfloat16
STAGE = int(os.environ.get("STAGE", "9"))


def _bc_t(t2d):
    """[128, X] tile -> [128, T, X] AP, repeated across t (middle dim)."""
    return bass.AP(
        tensor=t2d.tensor, offset=t2d.offset, ap=[t2d.ap[0], [0, T], list(t2d.ap[-1])]
    )


def _swap_free(t3d):
    """[128, T, G] tile -> [128, G, T] AP view (free dims swapped)."""
    return bass.AP(
        tensor=t3d.tensor,
        offset=t3d.offset,
        ap=[t3d.ap[0], list(t3d.ap[2]), list(t3d.ap[1])],
    )


def _as3d(t2d):
    """[128, SCW] dense tile -> [128, T, G] AP view."""
    return bass.AP(
        tensor=t2d.tensor, offset=t2d.offset, ap=[t2d.ap[0], [G, T], [1, G]]
    )


def build_nc():
    nc = bacc.Bacc("TRN2", target_bir_lowering=False, debug=False)

    predC_d = nc.declare_dram_parameter("predC", [NIMG, 4, P_TOT], BF16, isOutput=False)
    predW_d = nc.declare_dram_parameter("predW", [NIMG, 6, P_TOT], F32, isOutput=False)
    predA_d = nc.declare_dram_parameter("predA", [NIMG, 2, P_TOT], BF16, isOutput=False)
    tbM_d = nc.declare_dram_parameter("tbM", [NIMG, 4, SCW], BF16, isOutput=False)
    tbqD_d = nc.declare_dram_parameter("tbqD", [NIMG, 48, SCW], F32, isOutput=False)
    sR_d = nc.declare_dram_parameter("sR", [NIMG, 16, SCW], BF16, isOutput=False)
    logits_d = nc.declare_dram_parameter("logits", [NIMG, C], F32, isOutput=False)
    oh80_d = nc.declare_dram_parameter("oh80", [NIMG, C], F32, isOutput=False)
    out_d = nc.declare_dram_parameter("out", [NP, 66], F32, isOutput=True)

    with tile.TileContext(nc) as tc:
        consts = tc.alloc_tile_pool(name="consts", bufs=1)
        imgp = tc.alloc_tile_pool(name="imgp", bufs=2)
        chkp = tc.alloc_tile_pool(name="chkp", bufs=3)
        psp = tc.alloc_tile_pool(name="psp", bufs=2, space="PSUM")

        out_sb = consts.tile([NP, 66], F32)
        nc.vector.memset(out_sb, 0.0)

        # ---------------- classification loss (tiny) ----------------
        logit_sb = consts.tile([NIMG, C], F32)
        nc.sync.dma_start(out=logit_sb, in_=logits_d.ap())
        oh_sb = consts.tile([NIMG, C], F32)
        nc.sync.dma_start(out=oh_sb, in_=oh80_d.ap())

        mx = consts.tile([NIMG, 1], F32)
        nc.vector.tensor_reduce(mx, logit_sb, AXX, ALU.max)
        zc = consts.tile([NIMG, C], F32)
        nc.vector.tensor_scalar(zc, logit_sb, mx, None, ALU.subtract)
        ez = consts.tile([NIMG, C], F32)
        se = consts.tile([NIMG, 1], F32)
        nc.scalar.activation(ez, zc, ACTF.Exp, accum_out=se)
        lnse = consts.tile([NIMG, 1], F32)
        nc.scalar.activation(lnse, se, ACTF.Ln)
        zl = consts.tile([NIMG, 1], F32)
        zprod = consts.tile([NIMG, C], F32)
        nc.vector.tensor_tensor(zprod, zc, oh_sb, ALU.mult)
        nc.vector.tensor_reduce(zl, zprod, AXX, ALU.add)
        # nll = lnse - (z_label - mx) = lse - z_label
        nc.vector.tensor_tensor(out_sb[0:NIMG, 64:65], lnse, zl, ALU.subtract)

        # ---------------- bbox loss ----------------
        for img in range(NIMG):
            # per-image constant tiles
            predC4 = imgp.tile([NP, 4, NCH], BF16, tag="predC4")
            src = bass.AP(
                tensor=predC_d.ap().tensor,
                offset=predC_d.ap().offset + img * 4 * P_TOT,
                ap=[[1, NP], [P_TOT, 4], [NP, NCH]],
            )
            nc.sync.dma_start(out=predC4, in_=src)

            tbm = []
            for coord in range(4):
                btile2 = imgp.tile([NP, SCW], BF16, tag=f"tbm{coord}")
                src = bass.AP(
                    tensor=tbM_d.ap().tensor,
                    offset=tbM_d.ap().offset + (img * 4 + coord) * SCW,
                    ap=[[0, NP], [1, SCW]],
                )
                nc.gpsimd.dma_start(out=btile2, in_=src)
                tbm.append(_as3d(btile2))
            tx1m, ty1m, tx2m, ty2m = tbm

            tbqD_sb = imgp.tile([48, SCW], F32, tag="tbqD")
            nc.sync.dma_start(out=tbqD_sb, in_=tbqD_d.ap()[img])
            sR_sb = imgp.tile([16, SCW], BF16, tag="sR")
            nc.sync.dma_start(out=sR_sb, in_=sR_d.ap()[img])

            for sc in range(NSC):
                c0 = sc * G
                col = img * NSC + sc

                def predw(coord):
                    t2 = predC4[:, coord, c0 : c0 + G]
                    return _bc_t(t2)

                # PE: u = |pred - tb|^2 (f32), s = pa + ta (bf16 in, f32 psum)
                if STAGE < 3:
                    continue
                wU = chkp.tile([48, NP], F32, tag="wU")
                srcU = bass.AP(
                    tensor=predW_d.ap().tensor,
                    offset=predW_d.ap().offset + img * 6 * P_TOT + c0 * NP,
                    ap=[[NP, G], [P_TOT, 6], [1, NP]],
                )
                nc.sync.dma_start(out=wU, in_=srcU)
                u_ps = psp.tile([NP, SCW], F32, tag="u")
                H = SCW // 2
                nc.tensor.matmul(
                    u_ps[:, 0:H], wU, tbqD_sb[:, 0:H], start=True, stop=True
                )
                nc.tensor.matmul(
                    u_ps[:, H:SCW], wU, tbqD_sb[:, H:SCW], start=True, stop=True
                )

                wS = chkp.tile([16, NP], BF16, tag="wS")
                srcS = bass.AP(
                    tensor=predA_d.ap().tensor,
                    offset=predA_d.ap().offset + img * 2 * P_TOT + c0 * NP,
                    ap=[[NP, G], [P_TOT, 2], [1, NP]],
                )
                nc.sync.dma_start(out=wS, in_=srcS)
                s_ps = psp.tile([NP, SCW], F32, tag="s")
                nc.tensor.matmul(
                    s_ps[:, 0:H], wS, sR_sb[:, 0:H], start=True, stop=True
                )
                nc.tensor.matmul(
                    s_ps[:, H:SCW], wS, sR_sb[:, H:SCW], start=True, stop=True
                )

                if STAGE < 4:
                    continue
                uSB = chkp.tile([NP, T, G], BF16, tag="uSB")
                nc.scalar.activation(uSB, _as3d(u_ps), ACTF.Copy)
                sSB = chkp.tile([NP, T, G], BF16, tag="sSB")
                nc.scalar.activation(sSB, _as3d(s_ps), ACTF.Copy)

                # intervals
                if STAGE < 5:
                    continue
                mnx = chkp.tile([NP, T, G], BF16, tag="mnx")
                nc.vector.tensor_tensor(mnx, tx2m, predw(2), ALU.min)
                mxx = chkp.tile([NP, T, G], BF16, tag="mxx")
                nc.vector.tensor_tensor(mxx, tx1m, predw(0), ALU.max)
                dx = chkp.tile([NP, T, G], BF16, tag="dx")
                nc.gpsimd.tensor_tensor(dx, mnx, mxx, ALU.subtract)

                mny = chkp.tile([NP, T, G], BF16, tag="mny")
                nc.vector.tensor_tensor(mny, ty2m, predw(3), ALU.min)
                mxy = chkp.tile([NP, T, G], BF16, tag="mxy")
                nc.vector.tensor_tensor(mxy, ty1m, predw(1), ALU.max)
                dy = chkp.tile([NP, T, G], BF16, tag="dy")
                nc.gpsimd.tensor_tensor(dy, mny, mxy, ALU.subtract)
                rdy = chkp.tile([NP, T, G], BF16, tag="rdy")
                nc.scalar.activation(rdy, dy, ACTF.Relu)

                # inter = relu(dx) * rdy   (fused)
                inter = chkp.tile([NP, T, G], BF16, tag="inter")
                nc.vector.scalar_tensor_tensor(
                    inter, dx, 0.0, rdy, ALU.max, ALU.mult
                )

                # key = i16(inter) - i16(s); rmax per chunk
                if STAGE < 6:
                    continue
                # clear inter's sign bit (-0.0 would wrap the int16 key)
                interP = chkp.tile([NP, T, G // 2], I32, tag="interP")
                nc.vector.tensor_scalar(
                    interP, inter.bitcast(I32), 0x7FFF7FFF, None, ALU.bitwise_and
                )
                key = chkp.tile([NP, T, G], I16, tag="key")
                nc.vector.tensor_tensor(
                    key, interP.bitcast(I16), sSB.bitcast(I16), ALU.subtract
                )
                # max over t via a dense-slice tree (reduce over middle dim
                # with a transposed AP view miscompiles)
                m1 = chkp.tile([NP, 64, G], I16, tag="m1")
                nc.vector.tensor_tensor(
                    m1, key[:, 0:64, :], key[:, 64:128, :], ALU.max
                )
                m2 = chkp.tile([NP, 32, G], I16, tag="m2")
                nc.vector.tensor_tensor(m2, m1[:, 0:32, :], m1[:, 32:64, :], ALU.max)
                m3 = chkp.tile([NP, 16, G], I16, tag="m3")
                nc.vector.tensor_tensor(m3, m2[:, 0:16, :], m2[:, 16:32, :], ALU.max)
                m4 = chkp.tile([NP, 8, G], I16, tag="m4")
                nc.vector.tensor_tensor(m4, m3[:, 0:8, :], m3[:, 8:16, :], ALU.max)
                m5 = chkp.tile([NP, 4, G], I16, tag="m5")
                nc.vector.tensor_tensor(m5, m4[:, 0:4, :], m4[:, 4:8, :], ALU.max)
                m6 = chkp.tile([NP, 2, G], I16, tag="m6")
                nc.vector.tensor_tensor(m6, m5[:, 0:2, :], m5[:, 2:4, :], ALU.max)
                rmax = chkp.tile([NP, 1, G], I16, tag="rmax")
                nc.vector.tensor_tensor(rmax, m6[:, 0:1, :], m6[:, 1:2, :], ALU.max)
                rmax = rmax[:, 0, :]

                if STAGE < 7:
                    continue
                # threshold-saturate: below-threshold chunks match nothing
                mth = chkp.tile([NP, G], I16, tag="mth")
                nc.vector.tensor_scalar(mth, rmax, float(ITH16), None, ALU.is_gt)
                rmaxP = chkp.tile([NP, G], I16, tag="rmaxP")
                nc.vector.memset(rmaxP, 32767)
                nc.vector.copy_predicated(rmaxP, mth, rmax)

                if STAGE < 8:
                    continue
                # fused selection: oh = (key == rmaxP), count; ohu = oh*u, sum
                oh_t = chkp.tile([NP, T, G], BF16, tag="oh")
                nc.vector.tensor_tensor_reduce(
                    oh_t,
                    key,
                    _bc_t(rmaxP).bitcast(I16),
                    1.0,
                    0.0,
                    ALU.is_equal,
                    ALU.add,
                    out_sb[:, 32 + col : 33 + col],
                    opt_aps=False,
                )
                ohu = chkp.tile([NP, T, G], BF16, tag="ohu")
                nc.vector.tensor_tensor_reduce(
                    ohu,
                    oh_t,
                    uSB,
                    1.0,
                    0.0,
                    ALU.mult,
                    ALU.add,
                    out_sb[:, col : col + 1],
                    opt_aps=False,
                )

        nc.sync.dma_start(out=out_d.ap(), in_=out_sb)

        for p in (psp, chkp, imgp, consts):
            p.release()

    nc.compile()
    return nc


_NC_CACHE = None


def _get_nc():
    global _NC_CACHE
    if _NC_CACHE is None:
        _NC_CACHE = build_nc()
    return _NC_CACHE


def make_in_maps(pred_bboxes, pred_classes, true_bboxes, true_labels):
    pred_bboxes = np.ascontiguousarray(pred_bboxes, dtype=np.float32)
    true_bboxes = np.ascontiguousarray(true_bboxes, dtype=np.float32)
    logits0 = np.ascontiguousarray(pred_classes[:, 0, :], dtype=np.float32)
    lab0 = np.asarray(true_labels)[:, 0].astype(np.int64)
    oh80 = np.zeros((B, C), dtype=np.float32)
    oh80[np.arange(B), lab0] = 1.0

    in_maps = []
    for c in range(NCORES):
        s = slice(c * NIMG, (c + 1) * NIMG)
        pred = pred_bboxes[s]                      # [2, P, 4]
        predT = np.ascontiguousarray(pred.transpose(0, 2, 1))  # [2, 4, P]
        P2 = (pred ** 2).sum(-1)                   # [2, P]
        ones_p = np.ones_like(P2)
        predW = np.concatenate(
            [-2.0 * predT, P2[:, None, :], ones_p[:, None, :]], axis=1
        ).astype(np.float32)                       # [2, 6, P]
        pa = np.clip(
            (predT[:, 2] - predT[:, 0]) * (predT[:, 3] - predT[:, 1]), 0.0, None
        )                                          # [2, P]
        predA = np.stack([pa, ones_p], axis=1).astype(BF)  # [2, 2, P]

        tb = true_bboxes[s]                        # [2, T, 4]
        tbT = np.ascontiguousarray(tb.transpose(0, 2, 1))  # [2, 4, T]
        ta = (tbT[:, 2] - tbT[:, 0]) * (tbT[:, 3] - tbT[:, 1])  # [2, T]
        qB = (tb ** 2).sum(-1)                     # [2, T]
        tbM = np.repeat(tbT, G, axis=-1).astype(BF)  # [2, 4, T*G]
        tbq = np.concatenate(
            [tbT, np.ones_like(qB)[:, None, :], qB[:, None, :]], axis=1
        )                                          # [2, 6, T]
        tbqD = np.zeros((NIMG, 48, SCW), dtype=np.float32)
        sRf = np.zeros((NIMG, 16, SCW), dtype=np.float32)
        for g in range(G):
            tbqD[:, 6 * g : 6 * g + 6, g::G] = tbq
            sRf[:, 2 * g, g::G] = 1.0
            sRf[:, 2 * g + 1, g::G] = ta

        in_maps.append(
            {
                "predC": predT.astype(BF),
                "predW": predW,
                "predA": predA,
                "tbM": tbM,
                "tbqD": tbqD,
                "sR": sRf.astype(BF),
                "logits": logits0[s],
                "oh80": oh80[s],
            }
        )
    return in_maps


def combine(outs):
    bbox_sum = 0.0
    n_matched = 0.0
    cls_sum = 0.0
    for o in outs:
        o64 = o.astype(np.float64)
        bbox_sum += o64[:, 0 : NIMG * NSC].sum()
        n_matched += o64[:, 32 : 32 + NIMG * NSC].sum()
        cls_sum += o64[0:NIMG, 64].sum()
    bbox_loss = 0.5 * bbox_sum / max(4.0 * n_matched, 1.0)
    cls_loss = cls_sum / B
    return np.float32(bbox_loss + cls_loss)


def run_device(in_maps, trace=False, **kwargs):
    nc = _get_nc()
    return run_bass_kernel_spmd(
        nc, in_maps, list(range(NCORES)), trace=trace, **kwargs
    )


def kernel(pred_bboxes, pred_classes, true_bboxes, true_labels):
    in_maps = make_in_maps(pred_bboxes, pred_classes, true_bboxes, true_labels)
    res = run_device(in_maps)
    outs = [res.results[i]["out"] for i in range(NCORES)]
    return combine(outs)
